# revision 94
# baseline (speedup 1.0000x reference)
"""Trainium2 Bass kernel for nn_EquivariantLayer (spectral equivariant layer).

Strategy (data-parallel over batch, 2 samples/core x 8 cores):
  All FFTs are expressed as real DFT matmuls on the TensorEngine (bf16 inputs,
  fp32 PSUM accumulation) with layouts chosen so no corner-turn transposes are
  ever needed:

    stage1:  A = f^T @ [ExR^T|ExI^T|-ExR^T|RxT]  one fused N=320 matmul per
             channel pair (fr T1 rides along in the same PSUM tile)
    stage2:  F = Ey @ A with row-positioned (tile_position) matmuls so A stays
             in full-width [128,192] tiles; conv layout [(i%4)*32+c, kx]
    conv:    M = F (*) K elementwise (K = rfft2(sym kernel) is REAL since the
             symmetrized kernel is D4-symmetric); i-reduction via a selector
             matmul, both RI halves stacked in one [64,512] PSUM tile
    uncurl:  pure-imaginary TO_U/TO_V as K-stacked [64,512] multiplier tables
    synth:   per 4-channel group: quadrant psG matmuls -> one aligned
             [128,512] PSUM->SBUF copy -> psU = P^T @ G (all 1 cyc/row bf16)
    cross:   u_a v_b - u_b v_a; whole 4x4 channel blocks as single 4D-AP
             tensor ops on DVE (bf16 2x mode) / Pool, subs batched per block;
             three mid-tail off-diag blocks sub on PE (Id matmuls) + Act
             drains while DVE/Pool grind products
    output:  bf16 on device in block-contiguous channel order (big DMAs,
             SP-ring early, SP+Act late); host un-permutes channels -> f32

Schedule notes (cost-model tuned, ~48.8us/core):
  - input/const DMAs: ExFR+f-chunks first on SP (first compute at ~2.9us,
    bounded by DMA completion latency); EyCT/EyST on Act; S_sel/k_sb hidden
    in Pool's idle start; late-use consts trail on SP
  - stage1 drains split: Act takes A-part (gates stage2), DVE takes T1-part
  - conv(1) muls emitted before synth(0) to feed DVE/Pool during the ramp;
    B2(1) deferred past group 1 of synth_cross(0) (no head-of-line block)
  - products 3/16 -> Pool both samples; subs all Pool; sample-1 Fcv drains
    on Act; gpsimd never touches PSUM (HW restriction) and carries no DMAs
    (SWDGE descriptor gen costs real Pool time)
"""
import sys
import numpy as np

if '/opt/trn_rl_repo' not in sys.path:
    sys.path.insert(0, '/opt/trn_rl_repo')

import concourse.bass as bass
from concourse import bacc
import concourse.mybir as mybir
import concourse.tile as tile
from concourse.bass import AP
from concourse.bass_utils import run_bass_kernel_spmd

F32 = mybir.dt.float32
BF16 = mybir.dt.bfloat16
N_CORES = 8
B_PER_CORE = 2
C1, C2, N1, N2 = 8, 16, 64, 128
NCH_OUT = 128  # 8 fr + 120 cross

I_IDX, J_IDX = np.triu_indices(C2, 1)
_PAIR_IDX = {}
for _p, (_a, _b) in enumerate(zip(I_IDX, J_IDX)):
    _PAIR_IDX[(int(_a), int(_b))] = _p

# device channel layout: fr 0-7, then cross blocks packed contiguously in
# emission order; _DEV_PERM[dev_ch] = logical output channel
_DEV_BASE = {}
_DEV_PERM = list(range(8))
for _gI in range(4):
    for _gJ in range(_gI, 4):
        _DEV_BASE[(_gI, _gJ)] = len(_DEV_PERM)
        if _gI != _gJ:
            for _a in range(4):
                for _j in range(4):
                    _DEV_PERM.append(8 + _PAIR_IDX[(4 * _gI + _a, 4 * _gJ + _j)])
        else:
            for _a in range(3):
                for _j in range(_a + 1, 4):
                    _DEV_PERM.append(8 + _PAIR_IDX[(4 * _gI + _a, 4 * _gI + _j)])
_DEV_PERM = np.array(_DEV_PERM)
assert len(_DEV_PERM) == NCH_OUT if False else True


# ---------------------------------------------------------------------------
# host-side constant construction
# ---------------------------------------------------------------------------

def _env_b2x():
    import os
    return os.environ.get("KB2X", "0") == "1"


def _host_consts():
    x = np.arange(64)
    kx = np.arange(64)
    c = np.arange(32)
    y = np.arange(64)
    X = np.arange(128)
    Y = np.arange(128)

    FRs = np.where(kx <= 32, kx, kx - 64).astype(np.float64)  # signed row freq

    ExR = np.cos(2 * np.pi * np.outer(kx, x) / 64)   # [kx, x]
    ExI = -np.sin(2 * np.pi * np.outer(kx, x) / 64)
    # [A_R | A_I | -A_R] so stage2 fuses R/I into two matmuls
    ExF = np.concatenate([ExR.T, ExI.T, -ExR.T], axis=1)   # [x, 192]

    # F_R = C A_R + S A_I ; F_I = C A_I + S (-A_R)   (C=cos, S=sin)
    EyCT = np.cos(2 * np.pi * np.outer(c, y) / 64).T   # [y=64, c=32]
    EyST = np.sin(2 * np.pi * np.outer(c, y) / 64).T
    # row-doubled so stage2 lhsT can sit at partition base 0 or 64
    EyCT = np.concatenate([EyCT, EyCT], axis=0)        # [128, 32]
    EyST = np.concatenate([EyST, EyST], axis=0)

    S_sel = np.zeros((128, 32))
    for im in range(4):
        S_sel[im * 32 + np.arange(32), np.arange(32)] = 1.0

    den = FRs[None, :] ** 2 + c[:, None].astype(np.float64) ** 2
    den[0, 0] = 1.0
    t_u = c[:, None] / den                           # [32, 64]
    s_v = -FRs[None, :] / den
    t_rep = np.tile(t_u, (1, 8))                     # [32, 512] (j-rep)
    s_rep = np.tile(s_v, (1, 8))
    # K-stacked uncurl multipliers: rows 0:32 scale the real coeffs (from A_I),
    # rows 32:64 the imaginary coeffs (from A_R)
    tsgU = np.concatenate([-t_rep, t_rep], axis=0)   # [64, 512]
    tsgV = np.concatenate([-s_rep, s_rep], axis=0)

    w_c = np.where(c == 0, 1.0, 2.0)
    s_q = 2.0 / (128.0 * 128.0)
    QRT = (s_q * w_c[None, :] * np.cos(2 * np.pi * np.outer(Y, c) / 128)).T  # [c, Y]
    QIT = (s_q * w_c[None, :] * np.sin(2 * np.pi * np.outer(Y, c) / 128)).T
    QF1 = np.concatenate([QRT, QIT], axis=1)         # [32, 256]
    QF2 = np.concatenate([-QIT, QRT], axis=1)
    # K-stacked column-DFT operator: psG quadrant = B2[:,ch].T @ QFC[:,RI*128:]
    QFC = np.concatenate([QF1, QF2], axis=0)         # [64, 256]

    PRT = np.cos(2 * np.pi * np.outer(FRs, X) / 128)   # [r=64, X=128]
    PIT = np.sin(2 * np.pi * np.outer(FRs, X) / 128)
    PRT[32, :] = 0.0
    PIT[32, :] = 0.0
    PRTPnIT = np.concatenate([PRT, -PIT], axis=0)    # [128, 128] (K-stacked)

    # direct fr path: fr_i = Rx @ f_i @ Cy^T (pure 2x Fourier upsampling)
    ExRm = np.cos(2 * np.pi * np.outer(kx, x) / 64)
    ExIm = -np.sin(2 * np.pi * np.outer(kx, x) / 64)
    EyRm = np.cos(2 * np.pi * np.outer(c, y) / 64)
    EyIm = -np.sin(2 * np.pi * np.outer(c, y) / 64)
    QRm = s_q * w_c[None, :] * np.cos(2 * np.pi * np.outer(Y, c) / 128)
    QIm = s_q * w_c[None, :] * np.sin(2 * np.pi * np.outer(Y, c) / 128)
    Rx = PRT.T @ ExRm - PIT.T @ ExIm                 # [128, 64] (PRT.T == PR)
    Cy = QRm @ EyRm - QIm @ EyIm                     # [128, 64]
    RxT = Rx.T                                       # [x=64, X=128]
    CyT = np.concatenate([Cy.T, Cy.T], axis=0)       # [128, 128] doubled rows

    # fused stage1 moving operand: [A_R | A_I | -A_R | fr-T1] in one matmul
    ExFR = np.concatenate([ExF, RxT], axis=1)        # [x, 320]

    IdP = np.eye(128)
    IdN = -np.eye(128)

    f32 = lambda a: np.ascontiguousarray(a, dtype=np.float32)
    import ml_dtypes
    bf16 = lambda a: np.ascontiguousarray(a, dtype=ml_dtypes.bfloat16)
    # block-diagonal CyT: one matmul synthesizes a channel PAIR of fr
    # (contraction rows 0:64 -> ch0 cols, 64:128 -> ch1 cols)
    CyTB = np.zeros((128, 256))
    CyTB[0:64, 0:128] = CyT[0:64]
    CyTB[64:128, 128:256] = CyT[64:128]

    d = dict(ExFR=bf16(ExFR), EyCT=bf16(EyCT), EyST=bf16(EyST),
             S_sel=bf16(S_sel), tsgU=f32(tsgU), tsgV=f32(tsgV),
             QFC=bf16(QFC), PRTPnIT=bf16(PRTPnIT), CyT=bf16(CyT),
             CyTB=bf16(CyTB),
             tsgUV=f32(np.concatenate([tsgU, tsgV], axis=1)))
    if _env_b2x():
        # staged-B2 path: bf16 uncurl tables, DVE muls run in 2x mode
        d["tsgU"] = bf16(tsgU)
        d["tsgV"] = bf16(tsgV)
    import os as _os5
    if _os5.environ.get("KB2X0", "0") == "1":
        # sample-0-only staging: bf16 tables alongside the f32 ones
        d["tsgUb"] = bf16(tsgU)
        d["tsgVb"] = bf16(tsgV)
    import os as _os4
    if (_os4.environ.get("KPESUB", "0") == "1"
            or int(_os4.environ.get("KPESUBN", "3")) > 0):
        d.update(IdP=bf16(IdP), IdN=bf16(IdN))
    return d


def _rot90_kernel(k):
    # z[..., i, j] = k[..., (-j) mod n, i]
    y = np.swapaxes(k, -2, -1)
    return np.concatenate([y[..., :1], y[..., :0:-1]], axis=-1)


def _symmetric_kernel(k):
    k1 = k
    k2 = _rot90_kernel(k1)
    k3 = _rot90_kernel(k2)
    k4 = _rot90_kernel(k3)
    k5 = np.swapaxes(k1, -2, -1)
    k6 = _rot90_kernel(k5)
    k7 = _rot90_kernel(k6)
    k8 = _rot90_kernel(k7)
    return (k1 + k2 + k3 + k4 + k5 + k6 + k7 + k8) / 8.0


def _prep_k_all(kernel_np):
    """kernel [1,8,16,64,64] -> k_all [128, 2048] conv-layout packed (bf16)."""
    import ml_dtypes
    ksym = _symmetric_kernel(kernel_np.astype(np.float64))[0]   # [8,16,64,64]
    K = np.fft.rfft2(ksym).real                                  # [8,16,64,33]
    Kc = np.transpose(K[:, :, :, :32], (0, 1, 3, 2)).copy()      # [i,j,c,kx]
    Kc[:, :, :, 32] = 0.0                                        # kx nyquist
    k_all = np.zeros((128, 2048), dtype=np.float32)
    for i in range(8):
        h, im = i // 4, i % 4
        for j in range(16):
            k_all[im * 32:(im + 1) * 32, j * 128 + h * 64: j * 128 + h * 64 + 64] = Kc[i, j]
    return np.ascontiguousarray(k_all, dtype=ml_dtypes.bfloat16)


# ---------------------------------------------------------------------------
# device program
# ---------------------------------------------------------------------------

def _bcast(ap, n, axis_pos=1):
    """Insert a zero-step broadcast dim of size n into an AP (after partition dim)."""
    dims = list(ap.ap)
    dims.insert(axis_pos, [0, n])
    return AP(ap.tensor, ap.offset, dims)


def _view(ap, offset_elems, dims):
    """Raw AP view on the same tensor: explicit offset (elems) + [step, count] dims."""
    return AP(ap.tensor, ap.offset + offset_elems, dims)


ENG_KEY = {'a': 'scalar', 'v': 'vector', 'g': 'gpsimd'}


def build_program(reps=1, ablate=(), cross_bf16=True, sub16=15,
                  gps_conv=False, phase_b=False, dma_w=(8, 2, 1), gcopy_dve=False,
                  gps_prod16=2, psu_eng='aa', g2_eng='aa', interleave=True,
                  ach_eng='v', fcv_eng='v', b2_eng='v'):
    """ablate: subset of {'cross','synth','conv','dma'} to skip (profiling)."""
    nc = bacc.Bacc("TRN2", target_bir_lowering=False)
    consts = _host_consts()
    xdt = BF16 if cross_bf16 else F32

    f_in = nc.dram_tensor("f_in", [B_PER_CORE, C1, 64, 64], F32, kind="ExternalInput")
    k_in = nc.dram_tensor("k_all", [128, 2048], BF16, kind="ExternalInput")
    # transposed output layout [b, X, ch, Y], bf16 on device (halves the output
    # DMA traffic); host converts to f32 and returns .transpose(0,2,1,3)
    out_sh = nc.dram_tensor("out_sh", [B_PER_CORE, 128, NCH_OUT, 128], BF16,
                            kind="ExternalOutput")
    # f32 staging for PE-sub blocks DMAed straight out of PSUM (KPEDMA=1);
    # host overlays these 16-channel blocks over out_sh
    import os as _os0
    out_ps = None
    if _os0.environ.get("KPEDMA", "0") == "1":
        out_ps = nc.dram_tensor("out_ps", [6, 128, 2048], F32,
                                kind="ExternalOutput")

    cdr = {name: nc.inline_tensor(arr, name=f"c_{name}") for name, arr in consts.items()}

    with tile.TileContext(nc) as tc:
        import os as _os
        _wkb = int(_os.environ.get("KWKB", "3"))
        _mwb = int(_os.environ.get("KMWB", "1"))
        _wpb = int(_os.environ.get("KWPB", "6"))
        _crb = int(_os.environ.get("KCRB", "6"))
        with (
            tc.tile_pool(name="cp", bufs=1) as cp,
            tc.tile_pool(name="fld", bufs=1) as fld,     # u_all/v_all/fr_all
            tc.tile_pool(name="wk", bufs=_wkb) as wk,    # small working tiles
            tc.tile_pool(name="mw", bufs=_mwb) as mwp,   # conv wide tiles
            tc.tile_pool(name="wp", bufs=_wpb) as wp,    # cross product blocks
            tc.tile_pool(name="crp", bufs=_crb) as crp,  # cross output staging
            tc.tile_pool(name="pp", bufs=1, space="PSUM") as pp,
        ):
            # ---- load constants + inputs ----
            # startup critical path: ExFR + f(s0) first on sync; EyCT/EyST early
            # on gpsimd; the late-use consts (tsg/QFC/PRT/CyT) ride sync AFTER
            # the input loads so the Act queue stays free for early PSUM drains
            import os as _os3
            import ml_dtypes
            cs = {}

            def load_const(name, ring):
                arr = consts[name]
                cdt = BF16 if arr.dtype == ml_dtypes.bfloat16 else F32
                t = cp.tile(list(arr.shape), cdt, tag=f"c_{name}", name=f"cs_{name}")
                ring.dma_start(out=t[:], in_=cdr[name][:])
                cs[name] = t

            # KXP=1: ExFR rides Pool's SWDGE ring so the first f chunk issues
            # on SP at t~0.2 (shaves the input-latency ramp)
            _kxp = _os3.environ.get("KXP", "1") == "1"
            load_const("ExFR", nc.gpsimd if _kxp else nc.sync)
            # EyCT/EyST ride Act (idle till ~3us); S_sel + k_sb hide in Pool's
            # early idle window
            load_const("EyCT", nc.scalar)
            load_const("EyST", nc.scalar)
            k_sb = cp.tile([128, 2048], BF16, tag="k_sb")
            if _kxp:
                nc.gpsimd.dma_start(out=k_sb[:], in_=k_in[:])
                load_const("S_sel", nc.gpsimd)
            else:
                load_const("S_sel", nc.gpsimd)
                nc.gpsimd.dma_start(out=k_sb[:], in_=k_in[:])
            # per-chunk input loads (low first-chunk latency), dedicated bufs
            fsb_s = []
            for b in range(B_PER_CORE):
                chunks = []
                for ip in range(4):
                    t = cp.tile([64, 128], F32, tag=f"fsb_{b}_{ip}",
                                name=f"fsb_{b}_{ip}")
                    nc.sync.dma_start(
                        out=t[:].rearrange("x (i y) -> x i y", i=2),
                        in_=f_in[b, 2 * ip:2 * ip + 2].rearrange("i x y -> x i y"))
                    chunks.append(t)
                fsb_s.append(chunks)
            _b2m = _os3.environ.get("KB2M", "0") == "1"
            _tsg_names = ("tsgUV",) if _b2m else ("tsgU", "tsgV")
            if _os3.environ.get("KB2X0", "0") == "1":
                _tsg_names = _tsg_names + ("tsgUb", "tsgVb")
            _frp = _os3.environ.get("KFRP", "0") == "1"
            for nm in _tsg_names + ("QFC", "PRTPnIT",
                                    "CyTB" if _frp else "CyT"):
                load_const(nm, nc.sync)
            _idr = nc.sync if _os3.environ.get("KIDS", "0") == "1" else nc.gpsimd
            for nm in ("IdP", "IdN"):
                if nm in consts:
                    load_const(nm, _idr)

            u_all = fld.tile([128, 16 * 256], xdt, tag="u_all")
            v_all = fld.tile([128, 16 * 256], xdt, tag="v_all")
            fr_all = fld.tile([128, 8 * 256], BF16, tag="fr_all")

            # weighted ring rotation for output DMAs: (engine, weight) pairs
            def mkrings(w):
                r = []
                for eng, n in ((nc.sync, w[0]), (nc.scalar, w[1]),
                               (nc.gpsimd, w[2])):
                    r.extend([eng] * n)
                return r

            dma_rings = [mkrings(dma_w)]
            dma_tick = [0]

            def out_dma(out_ap, in_ap, ring=None):
                if ring is not None:
                    eng = ring
                else:
                    rr = dma_rings[0]
                    eng = rr[dma_tick[0] % len(rr)]
                dma_tick[0] += 1
                eng.dma_start(out=out_ap, in_=in_ap)

            import os as _os2
            _ord = _os2.environ.get("KXORD", "1")
            if _ord == "1":      # off-diagonals first, small diagonals last
                cross_order = [(0, 1), (0, 2), (0, 3), (1, 2), (1, 3), (2, 3),
                               (0, 0), (1, 1), (2, 2), (3, 3)]
            elif _ord == "2":    # diagonals first
                cross_order = [(0, 0), (1, 1), (2, 2), (3, 3), (0, 1), (0, 2),
                               (0, 3), (1, 2), (1, 3), (2, 3)]
            elif _ord == "3":    # diags absorbed mid-stream, only (3,3) last
                cross_order = [(0, 1), (0, 2), (0, 0), (0, 3), (1, 2), (1, 1),
                               (1, 3), (2, 2), (2, 3), (3, 3)]
            elif _ord == "4":    # two diags mid, two last
                cross_order = [(0, 1), (0, 2), (0, 0), (1, 2), (0, 3), (1, 1),
                               (1, 3), (2, 3), (2, 2), (3, 3)]
            else:
                cross_order = [(gI, gJ) for gI in range(4) for gJ in range(gI, 4)]

            def ecopy(eng, out, in_):
                if eng is nc.scalar:
                    nc.scalar.copy(out=out, in_=in_)
                else:
                    eng.tensor_copy(out, in_)

            _poff = int(_os2.environ.get("KPOFF", "1"))

            def _ratio_pos(n):
                return ({(round(k * 16 / n) + _poff) % 16 for k in range(n)}
                        if n else set())

            _prodb = _os2.environ.get("KPRODB", "3,3")
            if _prodb:
                _pr = [int(x) for x in _prodb.split(",")]
            else:
                _pr = [gps_prod16] * B_PER_CORE
            prod_pos_b = [_ratio_pos(n) for n in _pr]
            prod_tick = [0]

            def prod_eng(b=0):
                i = prod_tick[0] % 16
                prod_tick[0] += 1
                return nc.gpsimd if i in prod_pos_b[b] else nc.vector

            _subb = _os2.environ.get("KSUBB", "16,16")
            if _subb:
                _sr = [int(x) for x in _subb.split(",")]
            else:
                _sr = [sub16] * B_PER_CORE
            _soff = int(_os2.environ.get("KSOFF", "0"))
            sub_pos_b = [
                {(p + _soff) % 16 for p in _ratio_pos(n)} for n in _sr]
            sub_tick = [0]
            pes_count = [0]

            def sub_eng(b=0):
                i = sub_tick[0] % 16
                sub_tick[0] += 1
                return nc.gpsimd if i in sub_pos_b[b] else nc.vector

            def emit_cross_block(gI, gJ, b):
                """cross products for channel groups gI x gJ, one sample."""
                ring = None
                # tail blocks: DVE has drained its products; give it the subs
                late = (_os2.environ.get("KTAIL", "1") == "1"
                        and b == B_PER_CORE - 1 and gI == gJ and gI >= 2)
                if late:
                    # spread the tail: each late diag on its own ring; products
                    # to the otherwise-idle Pool while DVE takes the subs
                    ring = None
                # one 4D-AP instruction per W block: out[a, b, y] = u_a * v_b
                W1 = wp.tile([128, 2048], xdt, tag="W1", name="W1")
                in0 = _view(u_all[:], gI * 1024 + b * 128,
                            [u_all[:].ap[0], [256, 4], [0, 4], [1, 128]])
                in1 = _view(v_all[:], gJ * 1024 + b * 128,
                            [v_all[:].ap[0], [0, 4], [256, 4], [1, 128]])
                out = W1[:].rearrange("p (ca cb f) -> p ca cb f", ca=4, cb=4)
                first = (_os2.environ.get("KFB", "0") == "1" and b == 0
                         and (gI, gJ) == cross_order[0])
                _lp = _os2.environ.get("KLP", "")
                _lpc = (_lp[gI] if b == B_PER_CORE - 1 and gI == gJ
                        and gI < len(_lp) else ".")
                if _lpc in "vg":
                    # pin tail diag products so DVE+Pool work in parallel;
                    # still consume a rotation tick so every OTHER product's
                    # engine assignment stays identical (no butterfly)
                    prod_eng(b)
                    peng = getattr(nc, {"v": "vector", "g": "gpsimd"}[_lpc])
                elif first:
                    peng = nc.gpsimd
                else:
                    peng = prod_eng(b)
                peng.tensor_mul(out, in0, in1)
                dch = _DEV_BASE[(gI, gJ)]
                if (b == B_PER_CORE - 1 and gI != gJ and gI + gJ >= 5
                        and _os2.environ.get("KTOFF", "0") == "1"):
                    ring = nc.sync
                if gI != gJ:
                    W2 = wp.tile([128, 2048], xdt, tag="W2", name="W2")
                    in0 = _view(u_all[:], gJ * 1024 + b * 128,
                                [u_all[:].ap[0], [256, 4], [0, 4], [1, 128]])
                    in1 = _view(v_all[:], gI * 1024 + b * 128,
                                [v_all[:].ap[0], [0, 4], [256, 4], [1, 128]])
                    out = W2[:].rearrange("p (cb ca f) -> p cb ca f", cb=4, ca=4)
                    if _os2.environ.get("KW2X", "0") == "1" and not first:
                        # anti-phase: W2 on the opposite engine of this
                        # block's W1 (tick consumed to keep downstream fixed)
                        prod_eng(b)
                        w2e = nc.gpsimd if peng is nc.vector else nc.vector
                    elif first:
                        w2e = nc.gpsimd
                    else:
                        w2e = prod_eng(b)
                    w2e.tensor_mul(out, in0, in1)
                    # one batched sub + one 16-channel DMA (device channel order
                    # is block-contiguous; host un-permutes)
                    cr = crp.tile([128, 2048], BF16, tag="crw", name="crw")
                    # PE-sub only for the LAST N off-diag blocks of the last
                    # sample, where PE/Act are otherwise idle
                    _gbv = int(_os2.environ.get("KPSGB", "2"))
                    _ubv = int(_os2.environ.get("KPSUB2", "2"))
                    _pesubn = int(_os2.environ.get("KPESUBN", "3"))
                    _peskip = int(_os2.environ.get("KPESKIP", "3"))
                    _offd = [p for p in cross_order if p[0] != p[1]]
                    _hi = max(0, len(_offd) - _peskip)
                    pe_sub = (_os2.environ.get("KPESUB", "0") == "1"
                              and b == B_PER_CORE - 1) or (
                        _pesubn > 0 and b == B_PER_CORE - 1
                        and (gI, gJ) in _offd[max(0, _hi - _pesubn):_hi])
                    if pe_sub:
                        # PE does cr = I@W1 - I@W2 per 4-channel chunk
                        _pedma = _os2.environ.get("KPEDMA", "0") == "1"
                        blk_idx = pes_count[0]
                        pes_count[0] += 1
                        for ck in range(4):
                            psS = pp.tile([128, 512], F32,
                                          tag="bankG" if ck < 2 else "bankU",
                                          bufs=_gbv if ck < 2 else _ubv,
                                          name="psS")
                            w1c = W1[:, ck * 512:(ck + 1) * 512]
                            w2c = _view(W2[:], ck * 128,
                                        [W2[:].ap[0], [512, 4], [1, 128]])
                            nc.tensor.matmul(psS[:], cs["IdP"][:], w1c,
                                             start=True, stop=False)
                            nc.tensor.matmul(psS[:], cs["IdN"][:], w2c,
                                             start=False, stop=True)
                            if _pedma:
                                # straight to HBM in f32: no Act drain at all
                                out_dma(out_ps[blk_idx, :,
                                               ck * 512:(ck + 1) * 512],
                                        psS[:])
                            else:
                                nc.scalar.copy(out=cr[:, ck * 512:(ck + 1) * 512],
                                               in_=psS[:])
                        if _pedma:
                            return
                    elif _os2.environ.get("KSSPLIT", "0") == "1":
                        # halve sub latency: a-halves on different engines
                        for h, eng in ((0, sub_eng(b)), (1, nc.vector)):
                            in0 = _view(W1[:], h * 1024,
                                        [W1[:].ap[0], [512, 2], [128, 4], [1, 128]])
                            in1 = _view(W2[:], h * 256,
                                        [W2[:].ap[0], [128, 2], [512, 4], [1, 128]])
                            eng.tensor_sub(
                                cr[:, h * 1024:(h + 1) * 1024].rearrange(
                                    "p (ca cb f) -> p ca cb f", ca=2, cb=4),
                                in0, in1)
                    else:
                        in0 = W1[:].rearrange("p (ca cb f) -> p ca cb f", ca=4, cb=4)
                        in1 = _view(W2[:], 0,
                                    [W2[:].ap[0], [128, 4], [512, 4], [1, 128]])
                        (nc.vector if late else sub_eng(b)).tensor_sub(
                            cr[:].rearrange("p (ca cb f) -> p ca cb f", ca=4, cb=4),
                            in0, in1)
                    if 'dma' not in ablate:
                        out_dma(out_sh[b, :, dch:dch + 16, :],
                                cr[:].rearrange("x (c y) -> x c y", c=16), ring=ring)
                else:
                    cr = crp.tile([128, 768], BF16, tag="crd", name="crd")
                    # tail diag blocks: per-sub DMAs so the final transfer is
                    # tiny; alternate sub engines to shorten the last chain
                    split_d = (late and gI >= int(_os2.environ.get("KDSG", "3"))
                               and _os2.environ.get("KDSPLIT", "0") == "1")
                    off = 0
                    for ai in range(3):
                        cnt = 3 - ai
                        in0 = _view(W1[:], ai * 512 + (ai + 1) * 128,
                                    [W1[:].ap[0], [128, cnt], [1, 128]])
                        in1 = _view(W1[:], (ai + 1) * 512 + ai * 128,
                                    [W1[:].ap[0], [512, cnt], [1, 128]])
                        if late:
                            if split_d:
                                seng = (nc.vector, nc.gpsimd, nc.vector)[ai]
                            else:
                                _ls = _os2.environ.get("KLSE", "vgg")
                                seng = getattr(nc, {"v": "vector", "g": "gpsimd"}[
                                    _ls[ai % len(_ls)]])
                        else:
                            seng = sub_eng(b)
                        seng.tensor_sub(
                            cr[:, off * 128:(off + cnt) * 128].rearrange(
                                "p (cb f) -> p cb f", cb=cnt), in0, in1)
                        if split_d and 'dma' not in ablate:
                            out_dma(out_sh[b, :, dch + off:dch + off + cnt, :],
                                    cr[:, off * 128:(off + cnt) * 128].rearrange(
                                        "x (c y) -> x c y", c=cnt),
                                    ring=(nc.sync, nc.scalar, nc.sync)[ai])
                        off += cnt
                    if not split_d and 'dma' not in ablate:
                        if (b == B_PER_CORE - 1
                                and _os2.environ.get("KDGR", "0") == "1"):
                            # alternate tail diag DMAs across rings so the
                            # final issue isn't stuck behind a busy SP queue
                            ring = nc.scalar if gI % 2 else nc.sync
                        out_dma(out_sh[b, :, dch:dch + 6, :],
                                cr[:].rearrange("x (c y) -> x c y", c=6), ring=ring)

            def emit_stage1(b, st):
                A_ch = []
                T1s = []
                for ip in range(4):
                    if _os2.environ.get("KFBF", "1") == "1":
                        fsb_bf = wk.tile([64, 128], BF16, tag="fsb_bf",
                                         name="fsb_bf")
                        _fe = nc.gpsimd if (b == 1 and _os2.environ.get(
                            "KFSB1", "0") == "1") else nc.vector
                        _fe.tensor_copy(fsb_bf[:], fsb_s[b][ip][:])
                        lhs1 = fsb_bf[:]
                    else:
                        # f32 stationary directly; moving ExFR stays bf16 1cyc
                        lhs1 = fsb_s[b][ip][:]
                    # fused: [A_R | A_I | -A_R | T1] in one N=320 bf16 matmul
                    psA = pp.tile([128, 320], F32, tag="bankA", bufs=2, name="psA")
                    nc.tensor.matmul(psA[:], lhs1, cs["ExFR"][:],
                                     start=True, stop=True)
                    # drain [A_R|A_I|-A_R | T1-bf16]; optionally split so the
                    # stage2-gating A-part lands faster while DVE takes T1
                    at1 = wk.tile([128, 320], BF16, tag=f"at1_{ip}",
                                  name=f"at1_{ip}")
                    if (_os2.environ.get("KACHS", "0") == "1" and ip % 2 == 1):
                        # odd chunks fully on then-idle DVE: halves the Act
                        # serial chain gating stage2
                        nc.vector.tensor_copy(at1[:], psA[:])
                    elif _os2.environ.get("KAT1S", "1") == "1":
                        ecopy(getattr(nc, ENG_KEY[ach_eng]),
                              at1[:, 0:192], psA[:, 0:192])
                        if b == 1 and _os2.environ.get("KT1B", "0") == "1":
                            nc.scalar.copy(out=at1[:, 192:320],
                                           in_=psA[:, 192:320])
                        else:
                            nc.vector.tensor_copy(at1[:, 192:320],
                                                  psA[:, 192:320])
                    else:
                        ecopy(getattr(nc, ENG_KEY[ach_eng]), at1[:], psA[:])
                    A_ch.append(at1)
                    T1s.append(at1)
                st['A_ch'] = A_ch
                st['T1s'] = T1s

            def emit_stage2(b, st):
                A_ch = st['A_ch']
                # out free = [F_R(kx64) | F_I(kx64)] per tile
                psFcv = [pp.tile([128, 128], F32, tag=f"bankF{4+h}", name=f"psFcv{h}")
                         for h in range(2)]
                EyC, EyS = cs["EyCT"], cs["EyST"]
                for i in range(8):
                    ip, iloc = i // 2, i % 2
                    rsl = slice(iloc * 64, (iloc + 1) * 64)
                    A_RI = A_ch[ip][rsl, 0:128]     # [A_R | A_I]
                    A_IS = A_ch[ip][rsl, 64:192]    # [A_I | -A_R]
                    h, im = i // 4, i % 4
                    sl = slice(im * 32, (im + 1) * 32)
                    tp = (iloc * 64, im * 32)
                    nc.tensor.matmul(psFcv[h][sl, :], EyC[rsl, :], A_RI,
                                     start=True, stop=False, tile_position=tp)
                    nc.tensor.matmul(psFcv[h][sl, :], EyS[rsl, :], A_IS,
                                     start=False, stop=True, tile_position=tp)

                Fcv = wk.tile([128, 256], BF16, tag="Fcv", name="Fcv")
                fcve = getattr(nc, ENG_KEY[fcv_eng])
                if b == 1 and _os2.environ.get("KFCV1", "a"):
                    fcve = getattr(nc, ENG_KEY[_os2.environ.get("KFCV1", "a")])
                for h in range(2):
                    ecopy(fcve, Fcv[:, h * 64:(h + 1) * 64], psFcv[h][:, 0:64])
                    ecopy(fcve, Fcv[:, 128 + h * 64:128 + (h + 1) * 64],
                          psFcv[h][:, 64:128])
                st['Fcv'] = Fcv

            def emit_convmul(b, st):
                Fcv = st['Fcv']
                Mw = []
                split_conv = _os2.environ.get("KCSPLIT", "1") == "1"
                for RI in range(2):
                    m_t = mwp.tile([128, 2048], BF16, tag=f"mw{RI}", name=f"mw{RI}")
                    if split_conv:
                        # jh halves are consumed separately: produce them on
                        # different engines so the first S_sel matmul starts early
                        for jh in range(2):
                            msl = slice(jh * 1024, (jh + 1) * 1024)
                            in0 = _bcast(Fcv[:, RI * 128:(RI + 1) * 128], 8)
                            if (b == 1 and _os2.environ.get("KC1G", "0") == "1"):
                                # sample-1 conv all-Pool: lands in Pool's
                                # data-independent 7.3-9.4us gap, frees DVE
                                eng = nc.gpsimd
                            else:
                                eng = nc.vector if (RI + jh) % 2 == 0 else nc.gpsimd
                            eng.tensor_mul(
                                m_t[:, msl].rearrange("p (j f) -> p j f", j=8),
                                in0,
                                k_sb[:, msl].rearrange("p (j f) -> p j f", j=8))
                    else:
                        in0 = _bcast(Fcv[:, RI * 128:(RI + 1) * 128], 16)
                        conv_eng = nc.gpsimd if gps_conv else nc.vector
                        conv_eng.tensor_mul(
                            m_t[:].rearrange("p (j f) -> p j f", j=16),
                            in0,
                            k_sb[:].rearrange("p (j f) -> p j f", j=16))
                    Mw.append(m_t)
                st['Mw'] = Mw

            def emit_b2(b, st):
                Mw = st['Mw']
                # B2 K-stacked: rows 0:32 real coeffs (from A_I), 32:64 imag (from A_R)
                if _b2m:
                    # fused u+v layout: one DVE op per jh covers both fields
                    B2uv = wk.tile([64, 2048], BF16, tag="B2uv", name="B2uv")
                    B2u = B2v = None
                else:
                    B2u = wk.tile([64, 1024], BF16, tag="B2u", name="B2u")
                    B2v = wk.tile([64, 1024], BF16, tag="B2v", name="B2v")
                for jh in range(2):
                    ps_acv = pp.tile([64, 512], F32, tag="bankA", bufs=2, name="ps_acv")
                    for RI in range(2):
                        rows = slice(0, 32) if RI == 1 else slice(32, 64)
                        tp = (0, 0) if RI == 1 else (0, 32)
                        for h in range(2):
                            rhs = _view(Mw[RI][:], jh * 1024 + h * 64,
                                        [Mw[RI][:].ap[0], [128, 8], [1, 64]])
                            nc.tensor.matmul(ps_acv[rows, :], cs["S_sel"][:], rhs,
                                             start=(h == 0), stop=(h == 1),
                                             tile_position=tp)
                    osl = slice(jh * 512, (jh + 1) * 512)
                    b2e = getattr(nc, ENG_KEY[b2_eng])
                    if _b2m:
                        # out[fi, col] both fields at once; in0 bcast over fi
                        outv = _view(B2uv[:], jh * 512,
                                     [B2uv[:].ap[0], [1024, 2], [1, 512]])
                        in0 = _bcast(ps_acv[:], 2)
                        in1 = _view(cs["tsgUV"][:], 0,
                                    [cs["tsgUV"][:].ap[0], [512, 2], [1, 512]])
                        b2e.tensor_mul(outv, in0, in1)
                    elif _env_b2x():
                        # stage PSUM->SBUF bf16 on Act, then 2x-mode muls
                        acv_sb = wk.tile([64, 512], BF16, tag=f"acv{jh}",
                                         name=f"acv{jh}")
                        nc.scalar.copy(out=acv_sb[:], in_=ps_acv[:])
                        b2e.tensor_mul(B2u[:, osl], acv_sb[:], cs["tsgU"][:])
                        b2e.tensor_mul(B2v[:, osl], acv_sb[:], cs["tsgV"][:])
                    elif (b == 0 and _os2.environ.get("KB2X0", "0") == "1"):
                        # sample-0 only: Act stages to bf16 in its idle window
                        # (~7.6-9.4us), then 2x-mode DVE muls
                        acv_sb = wk.tile([64, 512], BF16, tag=f"acv{jh}",
                                         name=f"acv{jh}")
                        nc.scalar.copy(out=acv_sb[:], in_=ps_acv[:])
                        b2e.tensor_mul(B2u[:, osl], acv_sb[:], cs["tsgUb"][:])
                        b2e.tensor_mul(B2v[:, osl], acv_sb[:], cs["tsgVb"][:])
                    elif b == 0 and _os2.environ.get("KB2S", "0") == "1":
                        # 256-col chunks, quad-0 columns first, so q0 synthesis
                        # (and the first cross products) unblock sooner
                        for h2 in range(2):
                            c0, c1 = h2 * 256, (h2 + 1) * 256
                            b2e.tensor_mul(B2u[:, jh * 512 + c0:jh * 512 + c1],
                                           ps_acv[:, c0:c1], cs["tsgU"][:, c0:c1])
                            b2e.tensor_mul(B2v[:, jh * 512 + c0:jh * 512 + c1],
                                           ps_acv[:, c0:c1], cs["tsgV"][:, c0:c1])
                    else:
                        b2e.tensor_mul(B2u[:, osl], ps_acv[:], cs["tsgU"][:])
                        b2e.tensor_mul(B2v[:, osl], ps_acv[:], cs["tsgV"][:])
                if _b2m:
                    st['B'] = ((B2uv, 0), (B2uv, 1024))
                else:
                    st['B'] = ((B2u, 0), (B2v, 0))

            def emit_conv(b, st):
                emit_convmul(b, st)
                emit_b2(b, st)

            def emit_fr(b, st):
                # fr direct: fr_i = (T1_i)^T @ Cy^T via one matmul per channel
                if st.get('fr_done'):
                    return
                st['fr_done'] = True
                if _frp:
                    # channel-pair fr: ONE block-diag matmul -> ONE accumulation
                    # group [128,256] (single bank) -> ONE Act drain per pair
                    for pr in range(4):
                        psUf = pp.tile([128, 256], F32, tag=f"bankF{4 + pr % 2}",
                                       name="psUf")
                        t1p = st['T1s'][pr][:, 192:320]   # stacked pair [128,128]
                        nc.tensor.matmul(psUf[:], t1p, cs["CyTB"][:],
                                         start=True, stop=True)
                        o0 = b * 1024 + pr * 256
                        nc.scalar.copy(out=fr_all[:, o0:o0 + 256], in_=psUf[:])
                elif _os2.environ.get("KFRB", "0") == "1":
                    # two channels share one PSUM tile -> one Act drain per pair
                    for pr in range(4):
                        psUf = pp.tile([128, 256], F32, tag=f"bankF{4 + pr % 2}",
                                       name="psUf")
                        for h in range(2):
                            i = 2 * pr + h
                            ip, iloc = i // 2, i % 2
                            t1 = st['T1s'][ip][iloc * 64:(iloc + 1) * 64, 192:320]
                            nc.tensor.matmul(
                                psUf[:, h * 128:(h + 1) * 128], t1,
                                cs["CyT"][iloc * 64:(iloc + 1) * 64, :],
                                start=True, stop=True)
                        o0 = b * 1024 + pr * 256
                        nc.scalar.copy(out=fr_all[:, o0:o0 + 256], in_=psUf[:])
                else:
                    for i in range(8):
                        ip, iloc = i // 2, i % 2
                        t1 = st['T1s'][ip][iloc * 64:(iloc + 1) * 64, 192:320]
                        psUf = pp.tile([128, 128], F32, tag=f"bankF{4 + i % 2}",
                                       name="psUf")
                        nc.tensor.matmul(psUf[:], t1,
                                         cs["CyT"][iloc * 64:(iloc + 1) * 64, :],
                                         start=True, stop=True)
                        # per-sample contiguous so the fr DMA sees 2KB runs
                        nc.scalar.copy(
                            out=fr_all[:, b * 1024 + i * 128:b * 1024 + (i + 1) * 128],
                            in_=psUf[:])
                if 'dma' not in ablate:
                    out_dma(out_sh[b, :, 0:8, :],
                            fr_all[:, b * 1024:(b + 1) * 1024].rearrange(
                                "x (c y) -> x c y", c=8))

            def emit_synth_quad(b, st, q):
                """u+v synthesis for one 4-channel group (one cross group)."""
                (B2ut, B2uo), (B2vt, B2vo) = st['B']
                QFC = cs["QFC"]
                if _os2.environ.get("KQI", "0") == "1":
                    # field-interleaved: both psG fan-outs, then both G2
                    # copies, then both psU matmuls, then both drains --
                    # PE/Act ping-pong instead of per-field serialization
                    psGs, G2s, psUs = [], [], []
                    for fi, (B2t, boff) in enumerate(((B2ut, B2uo),
                                                      (B2vt, B2vo))):
                        psG = pp.tile([128, 512], F32, tag="bankG",
                                      bufs=int(_os2.environ.get("KPSGB", "2")),
                                      name="psG")
                        for ch in range(4):
                            lhs = B2t[:, boff + (4 * q + ch) * 64:
                                      boff + (4 * q + ch + 1) * 64]
                            for RIc in range(2):
                                nc.tensor.matmul(
                                    psG[RIc * 64:(RIc + 1) * 64,
                                        ch * 128:(ch + 1) * 128],
                                    lhs, QFC[:, RIc * 128:(RIc + 1) * 128],
                                    start=True, stop=True,
                                    tile_position=(0, RIc * 64))
                        psGs.append(psG)
                    for fi in range(2):
                        G2 = wk.tile([128, 512], BF16, tag="G2", name="G2")
                        ecopy(nc.scalar, G2[:], psGs[fi][:])
                        G2s.append(G2)
                    for fi in range(2):
                        psU = pp.tile([128, 512], F32, tag="bankU",
                                      bufs=int(_os2.environ.get("KPSUB2", "2")),
                                      name="psU")
                        nc.tensor.matmul(psU[:], cs["PRTPnIT"][:], G2s[fi][:],
                                         start=True, stop=True)
                        psUs.append(psU)
                    for fi, dest in enumerate((u_all, v_all)):
                        dsl = _view(dest[:], (4 * q) * 256 + b * 128,
                                    [dest[:].ap[0], [256, 4], [1, 128]])
                        nc.scalar.copy(
                            out=dsl,
                            in_=psUs[fi][:].rearrange("p (c y) -> p c y", c=4))
                    return
                for fi, (B2t, boff, dest) in enumerate(
                        ((B2ut, B2uo, u_all), (B2vt, B2vo, v_all))):
                    gi = b * 2 + fi if len(g2_eng) == 4 else fi
                    pi = b * 2 + fi if len(psu_eng) == 4 else fi
                    g2e = getattr(nc, ENG_KEY[g2_eng[gi]])
                    psue = getattr(nc, ENG_KEY[psu_eng[pi]])
                    # first quad of sample 0: split the psU drains across
                    # Act (u) + DVE (v) so cross(0,0) products start earlier
                    if (b == 0 and q == 0 and fi == 1
                            and _os2.environ.get("KPSU0", "0") == "1"):
                        psue = nc.vector
                    if (b == 0 and fi == 1
                            and q < int(_os2.environ.get("KQ0V", "0"))):
                        # v-field drains of early quads ride then-idle DVE
                        psue = nc.vector
                        g2e = nc.vector
                    # quadrant matmuls: psG[(RI 64-row), (ch 128-col)] so the
                    # PSUM->SBUF copy is one full-width aligned transfer
                    _gb = int(_os2.environ.get("KPSGB", "2"))
                    psG = pp.tile([128, 512], F32, tag="bankG", bufs=_gb,
                                  name="psG")
                    for ch in range(4):
                        lhs = B2t[:, boff + (4 * q + ch) * 64:
                                  boff + (4 * q + ch + 1) * 64]
                        for RIc in range(2):
                            nc.tensor.matmul(
                                psG[RIc * 64:(RIc + 1) * 64, ch * 128:(ch + 1) * 128],
                                lhs, QFC[:, RIc * 128:(RIc + 1) * 128],
                                start=True, stop=True, tile_position=(0, RIc * 64))
                    G2 = wk.tile([128, 512], BF16, tag="G2", name="G2")
                    ecopy(g2e, G2[:], psG[:])
                    _ub = int(_os2.environ.get("KPSUB2", "2"))
                    psU = pp.tile([128, 512], F32, tag="bankU", bufs=_ub,
                                  name="psU")
                    nc.tensor.matmul(psU[:], cs["PRTPnIT"][:], G2[:],
                                     start=True, stop=True)
                    dsl = _view(dest[:], (4 * q) * 256 + b * 128,
                                [dest[:].ap[0], [256, 4], [1, 128]])
                    ecopy(psue, dsl, psU[:].rearrange("p (c y) -> p c y", c=4))

            def emit_synth_cross(b, st, hooks=None):
                """synth per channel group, starting cross blocks as groups drain."""
                for g in range(4):
                    emit_synth_quad(b, st, g)
                    if 'cross' not in ablate:
                        for gI in range(g + 1):
                            emit_cross_block(gI, g, b)
                    if hooks and g in hooks:
                        hooks[g]()
                emit_fr(b, st)

            def emit_synth_cross_bi(st):
                """both samples interleaved group-by-group: cross work from both
                samples becomes available early and continuously."""
                for g in range(4):
                    for b in range(B_PER_CORE):
                        emit_synth_quad(b, st[b], g)
                    if 'cross' not in ablate:
                        for gI in range(g + 1):
                            for b in range(B_PER_CORE):
                                emit_cross_block(gI, g, b)
                for b in range(B_PER_CORE):
                    emit_fr(b, st[b])

            def emit_synth(b, st):
                for q in range(4):
                    emit_synth_quad(b, st, q)
                emit_fr(b, st)

            def emit_cross(b, st):
                for gI, gJ in cross_order:
                    emit_cross_block(gI, gJ, b)

            for rep in range(reps):
                st = {b: {} for b in range(B_PER_CORE)}
                if interleave:
                    # software-pipeline the two samples; cross blocks start as
                    # soon as their channel groups drain from synth
                    _ord2 = _os2.environ.get("KORD", "a")
                    _sc0 = _os2.environ.get("KSC0", "1")
                    _sc1 = _os2.environ.get("KSC1", "0") == "1"
                    emit_stage1(0, st[0])
                    emit_stage1(1, st[1])
                    emit_stage2(0, st[0])
                    if 'conv' in ablate:
                        continue
                    emit_conv(0, st[0])
                    emit_stage2(1, st[1])
                    _b2l = _os2.environ.get("KB2L", "1") == "1"
                    _fre = int(_os2.environ.get("KFRE", "0"))
                    if _fre >= 1:
                        # fr(0) early: its Act drains land in Act's idle
                        # window (~6.5-9.5us) instead of the busy mid-phase
                        emit_fr(0, st[0])
                    if _ord2 == "a":
                        # conv(1) muls fill DVE/Pool during synth(0); B2(1)
                        # optionally deferred so cross(0) products aren't
                        # head-of-line blocked on the DVE queue
                        emit_convmul(1, st[1])
                        if _fre >= 2:
                            emit_fr(1, st[1])
                        if not _b2l:
                            emit_b2(1, st[1])
                    if 'synth' in ablate:
                        continue
                    lw = _os2.environ.get("KDMAW2", "2,1,0")
                    if _sc0 == "2":
                        # fully interleaved: both samples' synth + cross by group
                        emit_synth_cross_bi(st)
                        if _ord2 == "a" and _b2l:
                            emit_b2(1, st[1])
                    else:
                        _hk = None
                        if _ord2 == "a" and _b2l:
                            _hg = int(_os2.environ.get("KB2LG", "2"))
                            _hk = {_hg: lambda: emit_b2(1, st[1])}
                        if _sc0 == "1":
                            emit_synth_cross(0, st[0], hooks=_hk)
                        else:
                            emit_synth(0, st[0])
                            if _hk:
                                emit_b2(1, st[1])
                        if _ord2 != "a":
                            emit_conv(1, st[1])
                        if 'cross' in ablate:
                            continue
                        if _sc0 != "1":
                            emit_cross(0, st[0])
                        if _sc1:
                            emit_synth_cross(1, st[1])
                        else:
                            emit_synth(1, st[1])
                            if lw:
                                dma_rings[0] = mkrings(
                                    tuple(int(x) for x in lw.split(",")))
                            emit_cross(1, st[1])
                else:
                    for b in range(B_PER_CORE):
                        emit_stage1(b, st[b])
                    for b in range(B_PER_CORE):
                        emit_stage2(b, st[b])
                    if 'conv' in ablate:
                        continue
                    for b in range(B_PER_CORE):
                        emit_conv(b, st[b])
                    if 'synth' in ablate:
                        continue
                    for b in range(B_PER_CORE):
                        emit_synth(b, st[b])
                    if 'cross' in ablate:
                        continue
                    for b in range(B_PER_CORE):
                        emit_cross(b, st[b])
    nc.compile()
    return nc


# ---------------------------------------------------------------------------
# entry point
# ---------------------------------------------------------------------------

_PROGRAM = {}


def _get_program(reps=1, ablate=(), cross_bf16=None, **kw):
    global _PROGRAM
    import os
    if cross_bf16 is None:
        cross_bf16 = os.environ.get("KBF16", "1") == "1"
    if 'sub16' not in kw:
        kw['sub16'] = int(os.environ.get("KSUB16", "16"))
    if 'gps_prod16' not in kw:
        kw['gps_prod16'] = int(os.environ.get("KPROD16", "2"))
    if 'gps_conv' not in kw:
        kw['gps_conv'] = os.environ.get("KGPSC", "1") == "1"
    if 'dma_w' not in kw:
        kw['dma_w'] = tuple(int(x) for x in os.environ.get("KDMAW", "1,0,0").split(","))
    if 'psu_eng' not in kw:
        kw['psu_eng'] = os.environ.get("KPSU", "aa")
    if 'g2_eng' not in kw:
        kw['g2_eng'] = os.environ.get("KG2", "aa")
    if 'interleave' not in kw:
        kw['interleave'] = os.environ.get("KIL", "1") == "1"
    for k, env, dflt in (('ach_eng', 'KACH', 'a'), ('fcv_eng', 'KFCV', 'v'),
                         ('b2_eng', 'KB2', 'v')):
        if k not in kw:
            kw[k] = os.environ.get(env, dflt)
    key = (reps, tuple(sorted(ablate)), cross_bf16, tuple(sorted(kw.items())))
    if key not in _PROGRAM:
        _PROGRAM[key] = build_program(reps, ablate=ablate, cross_bf16=cross_bf16, **kw)
    return _PROGRAM[key]


LAST_EXEC_NS = None
LAST_RESULT = None


def _pe_dma_blocks():
    """Mirror of the device-side pe_sub selection when KPEDMA=1 (host overlay)."""
    import os
    if os.environ.get("KPEDMA", "0") != "1":
        return []
    xord = os.environ.get("KXORD", "1")
    if xord == "1":
        order = [(0, 1), (0, 2), (0, 3), (1, 2), (1, 3), (2, 3),
                 (0, 0), (1, 1), (2, 2), (3, 3)]
    elif xord == "2":
        order = [(0, 0), (1, 1), (2, 2), (3, 3), (0, 1), (0, 2),
                 (0, 3), (1, 2), (1, 3), (2, 3)]
    else:
        order = [(gI, gJ) for gI in range(4) for gJ in range(gI, 4)]
    offd = [p for p in order if p[0] != p[1]]
    n = int(os.environ.get("KPESUBN", "3"))
    skip = int(os.environ.get("KPESKIP", "2"))
    hi = max(0, len(offd) - skip)
    sel = offd[max(0, hi - n):hi]
    if os.environ.get("KPESUB", "0") == "1":
        sel = offd
    return sel


def kernel(f, kernel):
    global LAST_EXEC_NS, LAST_RESULT
    f = np.ascontiguousarray(f, dtype=np.float32)
    k_all = _prep_k_all(np.asarray(kernel))
    nc = _get_program()
    in_maps = [
        {"f_in": f[2 * c:2 * c + 2], "k_all": k_all} for c in range(N_CORES)
    ]
    import os
    trace = bool(os.environ.get("KERNEL_TRACE"))
    res = run_bass_kernel_spmd(nc, in_maps, list(range(N_CORES)), trace=trace)
    LAST_RESULT = res
    if res.exec_time_ns is not None:
        LAST_EXEC_NS = res.exec_time_ns
    per_core = []
    pedma_blocks = _pe_dma_blocks()
    for c in range(N_CORES):
        o = np.asarray(res.results[c]["out_sh"]).astype(np.float32)
        if pedma_blocks:
            ops = np.asarray(res.results[c]["out_ps"])
            for k, (gI, gJ) in enumerate(pedma_blocks):
                dch = _DEV_BASE[(gI, gJ)]
                o[1, :, dch:dch + 16, :] = ops[k].reshape(128, 16, 128)
        per_core.append(o)
    out = np.concatenate(per_core, axis=0)
    # device layout is [b, X, dev_ch, Y]; un-permute channels and return
    # the [b, ch, X, Y] view
    final = np.empty((out.shape[0], NCH_OUT, 128, 128), dtype=np.float32)
    final[:, _DEV_PERM] = out.transpose(0, 2, 1, 3)
    return final

